# revision 40
# baseline (speedup 1.0000x reference)
"""Trainium2 Bass kernel for nn_MixedResolutionCNN.

Network (per sample, eval mode):
  high branch: ridgelet conv 3->16 k=15 same-pad (kernel broadcast over in-ch)
               -> relu -> maxpool2 -> 4096 feats
  low branch:  bilinear resize 32->8 -> conv 3->4 k=3 pad1 + bias -> relu
               -> maxpool2 -> 64 feats
  head:        concat -> fc 4160->1024 relu -> 1024->256 relu -> 256->5

Device strategy (pure data parallel over 8 cores, 512 images/core):

* The ridgelet kernel is identical across the 3 input channels, so the high
  conv contracts the channel-summed image xs = sum_c x[:,c] with a 16x15x15
  kernel. Expressed as matmuls with contraction over (v, i') = (kernel col,
  image row): out[(o,i),(b,j)] = sum Khat[o, i'-i+7, v] * xs[b, i', j+v-7].
  The host pre-sums channels (fp32) and ships a skewed 4x replication:
  partition block dv holds xs shifted by dv columns, so the moving operand
  for v-chunk kc is the uniform-offset slice [4kc+4, +32) of each 48-col
  image block - matmul-ready straight off the DMA, no on-chip prep.
  4 K-chunks x 4 M-chunks of [128,128,512] matmuls per 16-image tile.
* PE moving-operand APs must be innermost-contiguous: strided rhs runs the
  PE at 1/4 rate (measured).  All layouts here are chosen so every rhs is
  either contiguous 512 or contiguous-32-in-48 blocks.
* relu/maxpool: relu on scalar from PSUM, 2x2 pool as elementwise maxes on
  vector (relu commutes with max). Pooled features land in (j2, t, b) free
  order so every FC1 rhs slice is contiguous; no transpose anywhere.
* low branch: resize+conv fold into one linear map [3072, 256], packed
  feature-major as 24 full-128 contraction chunks whose rhs are contiguous
  slices of the f-major image tile; pool crosses partition halves via one
  64KB SBUF-SBUF DMA realign.
* DMA ring scheduling: the first conv matmuls only need their own wc chunk
  (4 tiles, one per kc, spread across rings); tball/wlow/w2 are issued from
  the busy scalar engine inside the conv loop to delay them off the
  startup-critical HBM window; the FC1 weight stream (8.5MB bf16) runs
  double-buffered alternating gpsimd/scalar rings (one ring tops out
  ~126GB/s, the stream needs 154GB/s).
"""

import numpy as np
import ml_dtypes

import concourse.bass as bass
import concourse.tile as tile
from concourse import mybir
from concourse.bass_utils import run_bass_kernel_spmd

BF16NP = ml_dtypes.bfloat16
FP32 = mybir.dt.float32
BF16 = mybir.dt.bfloat16

B = 4096
NCORES = 8
BC = B // NCORES           # 512 images per core
TIMGS = 16                 # images per tile
NTILES = BC // TIMGS       # 32
KS = 15
OUT_CH = 16


# ---------------------------------------------------------------- host math
def _ridgelet_kernel(r_dirs, r_scales, r_pos):
    """[16,15,15] channel-shared ridgelet kernel, mirrors reference."""
    c = np.arange(KS, dtype=np.float32) - KS // 2
    x1 = c[:, None]
    x2 = c[None, :]
    d = np.asarray(r_dirs, np.float32)[:, None, None]
    s = np.asarray(r_scales, np.float32)[:, None, None]
    p = np.asarray(r_pos, np.float32)[:, None, None]
    t = (x1 * np.cos(d) + x2 * np.sin(d) - p) / s
    vals = np.exp(-t * t / 2.0) - 0.5 * np.exp(-t * t / 8.0)
    return vals.reshape(OUT_CH, 10, KS, KS).sum(axis=1)


def _resize_mat(in_size=32, out_size=8):
    """Row matrix of jax.image.resize(..., 'bilinear', antialias=True)."""
    scale = out_size / in_size
    inv = 1.0 / scale
    kscale = max(inv, 1.0)
    sample_f = (np.arange(out_size, dtype=np.float64) + 0.5) * inv - 0.5
    x = np.abs(sample_f[None, :] - np.arange(in_size, dtype=np.float64)[:, None])
    w = np.maximum(0.0, 1.0 - x / kscale)
    w = w / w.sum(axis=0, keepdims=True)
    return w.T.astype(np.float32)  # [out, in]


def build_weights(inputs):
    """All packed device arrays (shared across cores)."""
    khat = _ridgelet_kernel(inputs["r_dirs"], inputs["r_scales"], inputs["r_pos"])
    # padded to 16x16 so v=15 / u out-of-range index to a zero slot
    khat_p = np.zeros((OUT_CH, 16, 16), np.float32)
    khat_p[:, :KS, :KS] = khat

    # conv lhsT: wc[p=(dv,i'), kc*512 + ch*128 + wi]
    dvip = np.arange(128)
    dv = dvip // 32
    ip = dvip % 32
    m = np.arange(512)
    ch = m // 128
    wi = m % 128
    par = ch // 2          # i parity (0=even rows, 1=odd)
    oh = ch % 2            # o half
    o = oh * 8 + wi // 16
    i2 = wi % 16
    i = 2 * i2 + par
    wc = np.zeros((128, 2048), np.float32)
    u = ip[:, None] - i[None, :] + 7          # [128, 512]
    umask = (u >= 0) & (u < KS)
    uc = np.clip(u, 0, 15)
    for kc in range(4):
        v = 4 * kc + dv                        # [128]
        vals = khat_p[o[None, :], uc, np.clip(v, 0, 15)[:, None]]
        vals = np.where(umask, vals, 0.0)
        wc[:, kc * 512:(kc + 1) * 512] = vals

    # low branch: fold resize+conv into [3072, 256], packed feature-major
    # into 24 chunks of 128 so every matmul has a full contraction dim
    A = _resize_mat()
    Ash = np.zeros((3, 8, 32), np.float32)
    for dh in range(3):
        for ph in range(8):
            r = ph + dh - 1
            if 0 <= r < 8:
                Ash[dh, ph] = A[r]
    wlow = np.asarray(inputs["wlow"], np.float32)
    # D[c,i,w,o,ph,pw] = sum_{dh,dw} wlow[o,c,dh,dw] Ash[dh,ph,i] Ash[dw,pw,w]
    D = np.einsum("ocuv,upi,vqw->ciwopq", wlow, Ash, Ash).astype(np.float32)
    D2 = D.reshape(3072, 4, 8, 8)    # [f=(c,i,w), o, ph, pw]
    W2 = np.zeros((3072, 256), np.float32)
    for g, (pp_, qq) in enumerate([(0, 0), (0, 1), (1, 0), (1, 1)]):
        W2[:, g * 64:(g + 1) * 64] = D2[:, :, pp_::2, qq::2].reshape(3072, 64)
    wlowp = W2.reshape(24, 128, 256).transpose(1, 0, 2).reshape(128, 24 * 256)

    # FC1 reorder: kstep = j2*2 + chunk over high feats, kstep 32 = low
    w1 = np.asarray(inputs["w1"], np.float32)          # [1024, 4160]
    w1hi = w1[:, 64:].reshape(1024, 16, 16, 16)        # [n, o, i2, j2]
    w1r = np.zeros((33, 128, 1024), np.float32)
    for ks in range(32):
        j2, c = ks // 2, ks % 2
        blk = w1hi[:, 8 * c:8 * (c + 1), :, j2]        # [n, 8, 16]
        w1r[ks] = blk.reshape(1024, 128).T
    w1r[32, :64, :] = w1[:, :64].T

    w2 = np.asarray(inputs["w2"], np.float32)          # [256, 1024]
    w2r = np.zeros((128, 2048), np.float32)
    for kc in range(8):
        w2r[:, kc * 256:(kc + 1) * 256] = w2[:, kc * 128:(kc + 1) * 128].T
    w3 = np.asarray(inputs["w3"], np.float32)          # [5, 256]
    w3r = np.zeros((128, 10), np.float32)
    for kc in range(2):
        w3r[:, kc * 5:(kc + 1) * 5] = w3[:, kc * 128:(kc + 1) * 128].T

    b1r = np.asarray(inputs["b1"], np.float32).reshape(8, 128).T.copy()
    b2r = np.asarray(inputs["b2"], np.float32).reshape(2, 128).T.copy()
    b3r = np.asarray(inputs["b3"], np.float32)[:, None].copy()
    blowr = np.tile(np.repeat(np.asarray(inputs["blow"], np.float32), 16),
                    2)[:, None].copy()

    return {
        "wc": wc.astype(BF16NP),
        "wlow": wlowp.astype(BF16NP),
        "w1r": w1r.astype(BF16NP),
        "w2r": w2r.astype(BF16NP),
        "w3r": w3r.astype(BF16NP),
        "b1r": np.ascontiguousarray(b1r),
        "b2r": np.ascontiguousarray(b2r),
        "b3r": b3r,
        "blowr": blowr,
    }


def pack_x(x_core):
    """[512,3,32,32] f32 -> xt2 [128, 12288] bf16, feature-major.

    xt2[p, k*512 + n] = x[n, f] for f = k*128 + p, f = c*1024 + i*32 + w.
    Low-branch rhs chunk k is the contiguous slice [:, k*512:(k+1)*512]
    with a full 128-partition contraction.
    """
    xc = np.asarray(x_core, np.float32).astype(BF16NP)
    arr = xc.reshape(BC, 3072).T.reshape(24, 128, BC)
    return np.ascontiguousarray(arr.transpose(1, 0, 2).reshape(128, 24 * BC))


def pack_x_high(x_core):
    """Channel-summed, 4x skew-replicated high-branch input, matmul-ready.

    xh[dv*32+i, t*768 + b*48 + v] = sum_c x[t*16+b, c, i, v+dv-11]
    (zero outside the image).  The conv rhs for v-chunk kc is the
    uniform-offset slice starting at 4*kc+4 of each 48-block: image col
    j = jj + (4*kc+dv) - 7.  Host-side channel sum (in fp32) shrinks the
    stream 3x and removes the on-chip adds entirely.
    """
    xc = np.asarray(x_core, np.float32)
    xs = xc.sum(axis=1).reshape(NTILES, TIMGS, 32, 32)
    buf = np.zeros((4, 32, NTILES, TIMGS, 48), np.float32)
    xsit = xs.transpose(2, 0, 1, 3)          # [i, t, b, j]
    for dv in range(4):
        lo = 11 - dv
        buf[dv, :, :, :, lo:lo + 32] = xsit
    return np.ascontiguousarray(
        buf.reshape(128, NTILES * 768).astype(BF16NP))


# ---------------------------------------------------------------- bass build
_WAIT_CARRIERS = ("InstEventSemaphore", "InstNoOp",
                  "InstUnconditionalBranch", "InstCompareAndBranch")


def _legalize_waits(nc):
    """Split excess semaphore waits onto same-engine NoOp carriers.

    The walrus codegen used by the bass2jax path allows at most 1 attached
    wait on compute instructions and 2 on DMA; Tile sometimes emits more.
    Engines execute instructions in order, so a preceding NoOp carrying the
    extra waits is equivalent.
    """
    uid = 0
    for blk in nc.m.functions[0].blocks:
        insts = blk.instructions
        i = 0
        while i < len(insts):
            inst = insts[i]
            ty = type(inst).__name__
            si = inst.sync_info
            if si is None or ty in _WAIT_CARRIERS:
                i += 1
                continue
            waits = list(si.on_wait or [])
            limit = 1
            if len(waits) <= limit:
                i += 1
                continue
            extra, keep = waits[:-limit], waits[-limit:]
            for w in extra:
                nop = mybir.InstNoOp(
                    name=f"waitnop-{uid}", engine=inst.engine,
                    sync_info=mybir.SyncInfo(on_wait=[w], on_update=[]))
                uid += 1
                insts.insert(i, nop)
                i += 1
            inst.sync_info = mybir.SyncInfo(
                on_wait=keep, on_update=list(si.on_update or []))
            i += 1


def build_nc(skip_conv=False, skip_low=False, skip_fc1=False, skip_fc23=False):
    nc = bass.Bass()
    xt_d = nc.declare_dram_parameter("xt", [128, 24 * BC], BF16, isOutput=False)
    xth_d = nc.declare_dram_parameter("xth", [128, NTILES * 768], BF16, isOutput=False)
    wc_d = nc.declare_dram_parameter("wc", [128, 2048], BF16, isOutput=False)
    wlow_d = nc.declare_dram_parameter("wlow", [128, 24 * 256], BF16, isOutput=False)
    w1_d = nc.declare_dram_parameter("w1r", [33, 128, 1024], BF16, isOutput=False)
    w2_d = nc.declare_dram_parameter("w2r", [128, 2048], BF16, isOutput=False)
    w3_d = nc.declare_dram_parameter("w3r", [128, 10], BF16, isOutput=False)
    b1_d = nc.declare_dram_parameter("b1r", [128, 8], FP32, isOutput=False)
    b2_d = nc.declare_dram_parameter("b2r", [128, 2], FP32, isOutput=False)
    b3_d = nc.declare_dram_parameter("b3r", [5, 1], FP32, isOutput=False)
    bl_d = nc.declare_dram_parameter("blowr", [128, 1], FP32, isOutput=False)
    y_d = nc.declare_dram_parameter("y", [5, 512], FP32, isOutput=True)

    RELU = mybir.ActivationFunctionType.Relu

    with tile.TileContext(nc) as tc:
        with (
            tc.tile_pool(name="persist", bufs=1) as pp,
            tc.tile_pool(name="work", bufs=3) as wp,
            tc.tile_pool(name="w1pool", bufs=4) as w1p,
            # single rotating PSUM pool for every phase: pool-close
            # barriers between phases cost 2-3us each; one pool turns
            # them into per-slot WAR deps (all tiles are [128,512] f32)
            tc.tile_pool(name="allpsum", bufs=8, space="PSUM") as cps,
        ):
            # wc first: the first conv matmul only needs wc + xh[0].  The
            # 3MB tball / wlow / w2 are not needed until ~120us in, and
            # their transfers would steal HBM bandwidth from the xh
            # stream, so they are issued from the (busy) scalar engine
            # inside the conv loop to physically delay them.
            # one tile per kc chunk: tile-granular deps let the kc=0
            # matmuls start as soon as their own chunk lands
            wc_sb = [pp.tile([128, 512], BF16, tag=f"wc{q}", name=f"wc{q}")
                     for q in range(4)]
            for q, eng in enumerate((nc.gpsimd, nc.sync, nc.scalar, nc.scalar)):
                eng.dma_start(out=wc_sb[q][:],
                              in_=wc_d[:, q * 512:(q + 1) * 512])
            b1_sb = pp.tile([128, 8], FP32, tag="b1")
            nc.sync.dma_start(out=b1_sb[:], in_=b1_d[:])
            b2_sb = pp.tile([128, 2], FP32, tag="b2")
            nc.sync.dma_start(out=b2_sb[:], in_=b2_d[:])
            b3_sb = pp.tile([5, 1], FP32, tag="b3")
            nc.sync.dma_start(out=b3_sb[:], in_=b3_d[:])
            bl_sb = pp.tile([128, 1], FP32, tag="bl")
            nc.sync.dma_start(out=bl_sb[:], in_=bl_d[:])
            w3_sb = pp.tile([128, 10], BF16, tag="w3")
            nc.sync.dma_start(out=w3_sb[:], in_=w3_d[:])
            w2_sb = pp.tile([128, 2048], BF16, tag="w2")
            wlow_sb = pp.tile([128, 24 * 256], BF16, tag="wlow")
            tball = pp.tile([128, 24 * BC], BF16, tag="tball")

            ph0 = pp.tile([128, 8192], BF16, tag="ph0")
            ph1 = pp.tile([128, 8192], BF16, tag="ph1")
            xlow_sb = pp.tile([128, 512], BF16, tag="xlow")
            nc.vector.memset(xlow_sb[:], 0.0)
            # per-chunk tiles: tile-granular deps let FC2/FC3 start on
            # chunk 0 without waiting for every activate
            h1_sb = [pp.tile([128, 512], BF16, tag=f"h1_{i}", name=f"h1_{i}")
                     for i in range(8)]
            h2_sb = [pp.tile([128, 512], BF16, tag=f"h2_{i}", name=f"h2_{i}")
                     for i in range(2)]
            y_sb = pp.tile([5, 512], FP32, tag="ysb")

            # ---------------- conv + pool over 32 tiles
            if True:
                for t in range(0 if skip_conv else NTILES):
                    xh = wp.tile([128, 768], BF16, tag="xh", bufs=10)
                    nc.gpsimd.dma_start(
                        out=xh[:], in_=xth_d[:, t * 768:(t + 1) * 768])
                    # late-issued persistent loads (see note above)
                    if t in (8, 11, 14, 17):
                        q = (t - 8) // 3
                        sl = slice(q * 3072, (q + 1) * 3072)
                        nc.scalar.dma_start(out=tball[:, sl], in_=xt_d[:, sl])
                    elif t == 20:
                        nc.scalar.dma_start(out=wlow_sb[:], in_=wlow_d[:])
                    elif t == 23:
                        nc.scalar.dma_start(out=w2_sb[:], in_=w2_d[:])

                    xhv = xh[:].rearrange("p (b v) -> p b v", v=48)
                    cp = [cps.tile([128, 512], FP32, tag="ps", name=f"cp{t}_{i}") for i in range(4)]
                    for kc in range(4):
                        off = 4 * kc + 4
                        rhs = xhv[:, :, off:off + 32]
                        for mc in range(4):
                            nc.tensor.matmul(
                                cp[mc][:],
                                wc_sb[kc][:, mc * 128:(mc + 1) * 128],
                                rhs,
                                start=(kc == 0),
                                stop=(kc == 3),
                            )
                    # relu first (1-input PSUM read), then pool in SBUF bf16
                    s = [wp.tile([128, 512], BF16, tag=f"s{i}", name=f"s{t}_{i}")
                         for i in range(4)]
                    for i in range(4):
                        nc.scalar.activation(out=s[i][:], in_=cp[i][:], func=RELU)
                    m0 = wp.tile([128, 512], BF16, tag="m0")
                    m1 = wp.tile([128, 512], BF16, tag="m1")
                    nc.vector.tensor_max(out=m0[:], in0=s[0][:], in1=s[2][:])
                    nc.vector.tensor_max(out=m1[:], in0=s[1][:], in1=s[3][:])
                    # pooled feats land in (j2, t, b) free order so FC1 rhs
                    # slices are contiguous; iterate (j, b) with contiguous
                    # 16-elem b-runs on the write side
                    m0v = m0[:].rearrange("p (b j t) -> p j b t", j=16, t=2)
                    m1v = m1[:].rearrange("p (b j t) -> p j b t", j=16, t=2)
                    ph0v_o = ph0[:].rearrange(
                        "p (j t b) -> p t j b", t=NTILES, b=TIMGS)[:, t]
                    ph1v_o = ph1[:].rearrange(
                        "p (j t b) -> p t j b", t=NTILES, b=TIMGS)[:, t]
                    nc.vector.tensor_max(
                        out=ph0v_o, in0=m0v[:, :, :, 0], in1=m0v[:, :, :, 1])
                    nc.vector.tensor_max(
                        out=ph1v_o, in0=m1v[:, :, :, 0], in1=m1v[:, :, :, 1])

            # ---------------- low branch (contiguous rhs slices, m=128)
            if True:
                lp = [cps.tile([128, 512], FP32, tag="ps", name=f"lp{g}")
                      for g in range(2)]
                for wg in range(0 if skip_low else 24):
                    rhs = tball[:, wg * 512:(wg + 1) * 512]
                    for g in range(2):
                        nc.tensor.matmul(
                            lp[g][:],
                            wlow_sb[:, wg * 256 + g * 128: wg * 256 + (g + 1) * 128],
                            rhs, start=(wg == 0), stop=(wg == 23))
                # relu(x+bias) per group first, then pool in SBUF; the
                # final max crosses partition halves, so DMA-realign the
                # top half (TensorTensor needs equal base partitions)
                r = [wp.tile([128, 512], BF16, tag=f"r{g}", name=f"r{g}")
                     for g in range(2)]
                for g in range(2):
                    nc.scalar.activation(
                        out=r[g][:], in_=lp[g][:], func=RELU, bias=bl_sb[:, 0:1])
                q0 = wp.tile([128, 512], BF16, tag="q0")
                nc.vector.tensor_max(out=q0[:], in0=r[0][:], in1=r[1][:])
                qb = wp.tile([64, 512], BF16, tag="qb")
                nc.sync.dma_start(out=qb[:], in_=q0[64:128, :])
                nc.vector.tensor_max(
                    out=xlow_sb[0:64, :], in0=q0[0:64, :], in1=qb[:])

            # ---------------- FC1 (weights streamed)
            if True:
                fp = [cps.tile([128, 512], FP32, tag="ps", name=f"fp{i}") for i in range(8)]
                for ks in range(0 if skip_fc1 else 33):
                    w1t = w1p.tile([128, 1024], BF16, tag="w1t", bufs=12)
                    # alternate rings: one ring can't sustain the 154 GB/s
                    # the FC1 weight stream needs.  The sync ring is idle
                    # during conv and prefetches the odd half early; the
                    # (now 3x lighter) xh stream leaves it the bandwidth.
                    w1_eng = nc.gpsimd if ks % 2 == 0 else nc.sync
                    w1_eng.dma_start(out=w1t[:], in_=w1_d[ks])
                    if ks < 32:
                        j2, c = ks // 2, ks % 2
                        src = ph0 if c == 0 else ph1
                        rhs = src[:, j2 * 512:(j2 + 1) * 512]
                    else:
                        rhs = xlow_sb[:]
                    for mc in range(8):
                        nc.tensor.matmul(
                            fp[mc][:], w1t[:, mc * 128:(mc + 1) * 128], rhs,
                            start=(ks == 0), stop=(ks == 32))
                for mc in range(8):
                    nc.scalar.activation(
                        out=h1_sb[mc][:], in_=fp[mc][:],
                        func=RELU, bias=b1_sb[:, mc:mc + 1])

            # ---------------- FC2 + FC3
            if True:
                gp = [cps.tile([128, 512], FP32, tag="ps", name=f"gp{i}") for i in range(2)]
                for kc in range(0 if skip_fc23 else 8):
                    rhs = h1_sb[kc][:]
                    for mc in range(2):
                        nc.tensor.matmul(
                            gp[mc][:],
                            w2_sb[:, kc * 256 + mc * 128: kc * 256 + (mc + 1) * 128],
                            rhs, start=(kc == 0), stop=(kc == 7))
                for mc in range(2):
                    nc.scalar.activation(
                        out=h2_sb[mc][:], in_=gp[mc][:],
                        func=RELU, bias=b2_sb[:, mc:mc + 1])

                yp = cps.tile([128, 512], FP32, tag="ps", name="yp")
                for kc in range(2):
                    nc.tensor.matmul(
                        yp[0:5, :], w3_sb[:, kc * 5:(kc + 1) * 5],
                        h2_sb[kc][:],
                        start=(kc == 0), stop=(kc == 1))
                nc.vector.tensor_scalar_add(
                    out=y_sb[:], in0=yp[0:5, :], scalar1=b3_sb[:, 0:1])
                nc.sync.dma_start(out=y_d[:], in_=y_sb[:])

    _legalize_waits(nc)
    return nc


_NC = None
TRACE = False
LAST_RESULTS = None


def kernel(**inputs):
    global _NC, LAST_RESULTS
    w = build_weights(inputs)
    if _NC is None:
        _NC = build_nc()
    x = np.asarray(inputs["x"], np.float32)
    in_maps = []
    for c in range(NCORES):
        xs_c = x[c * BC:(c + 1) * BC]
        m = {"xt": pack_x(xs_c), "xth": pack_x_high(xs_c)}
        m.update(w)
        in_maps.append(m)
    res = run_bass_kernel_spmd(_NC, in_maps, list(range(NCORES)), trace=TRACE)
    LAST_RESULTS = res
    y = np.concatenate(
        [np.asarray(res.results[i]["y"], np.float32).T for i in range(NCORES)], axis=0)
    return y



# revision 41
# speedup vs baseline: 1.0129x; 1.0129x over previous
"""Trainium2 Bass kernel for nn_MixedResolutionCNN.

Network (per sample, eval mode):
  high branch: ridgelet conv 3->16 k=15 same-pad (kernel broadcast over in-ch)
               -> relu -> maxpool2 -> 4096 feats
  low branch:  bilinear resize 32->8 -> conv 3->4 k=3 pad1 + bias -> relu
               -> maxpool2 -> 64 feats
  head:        concat -> fc 4160->1024 relu -> 1024->256 relu -> 256->5

Device strategy (pure data parallel over 8 cores, 512 images/core):

* The ridgelet kernel is identical across the 3 input channels, so the high
  conv contracts the channel-summed image xs = sum_c x[:,c] with a 16x15x15
  kernel. Expressed as matmuls with contraction over (v, i') = (kernel col,
  image row): out[(o,i),(b,j)] = sum Khat[o, i'-i+7, v] * xs[b, i', j+v-7].
  The host pre-sums channels (fp32) and ships a skewed 4x replication:
  partition block dv holds xs shifted by dv columns, so the moving operand
  for v-chunk kc is the uniform-offset slice [4kc+4, +32) of each 48-col
  image block - matmul-ready straight off the DMA, no on-chip prep.
  4 K-chunks x 4 M-chunks of [128,128,512] matmuls per 16-image tile.
* PE moving-operand APs must be innermost-contiguous: strided rhs runs the
  PE at 1/4 rate (measured).  All layouts here are chosen so every rhs is
  either contiguous 512 or contiguous-32-in-48 blocks.
* relu/maxpool: relu on scalar from PSUM, 2x2 pool as elementwise maxes on
  vector (relu commutes with max). Pooled features land in (j2, t, b) free
  order so every FC1 rhs slice is contiguous; no transpose anywhere.
* low branch: resize+conv fold into one linear map [3072, 256], packed
  feature-major as 24 full-128 contraction chunks whose rhs are contiguous
  slices of the f-major image tile; pool crosses partition halves via one
  64KB SBUF-SBUF DMA realign.
* DMA ring scheduling: the first conv matmuls only need their own wc chunk
  (4 tiles, one per kc, spread across rings); tball/wlow/w2 are issued from
  the busy scalar engine inside the conv loop to delay them off the
  startup-critical HBM window; the FC1 weight stream (8.5MB bf16) runs
  double-buffered alternating gpsimd/scalar rings (one ring tops out
  ~126GB/s, the stream needs 154GB/s).
"""

import numpy as np
import ml_dtypes

import concourse.bass as bass
import concourse.tile as tile
from concourse import mybir
from concourse.bass_utils import run_bass_kernel_spmd

BF16NP = ml_dtypes.bfloat16
FP32 = mybir.dt.float32
BF16 = mybir.dt.bfloat16

B = 4096
NCORES = 8
BC = B // NCORES           # 512 images per core
TIMGS = 16                 # images per tile
NTILES = BC // TIMGS       # 32
KS = 15
OUT_CH = 16


# ---------------------------------------------------------------- host math
def _ridgelet_kernel(r_dirs, r_scales, r_pos):
    """[16,15,15] channel-shared ridgelet kernel, mirrors reference."""
    c = np.arange(KS, dtype=np.float32) - KS // 2
    x1 = c[:, None]
    x2 = c[None, :]
    d = np.asarray(r_dirs, np.float32)[:, None, None]
    s = np.asarray(r_scales, np.float32)[:, None, None]
    p = np.asarray(r_pos, np.float32)[:, None, None]
    t = (x1 * np.cos(d) + x2 * np.sin(d) - p) / s
    vals = np.exp(-t * t / 2.0) - 0.5 * np.exp(-t * t / 8.0)
    return vals.reshape(OUT_CH, 10, KS, KS).sum(axis=1)


def _resize_mat(in_size=32, out_size=8):
    """Row matrix of jax.image.resize(..., 'bilinear', antialias=True)."""
    scale = out_size / in_size
    inv = 1.0 / scale
    kscale = max(inv, 1.0)
    sample_f = (np.arange(out_size, dtype=np.float64) + 0.5) * inv - 0.5
    x = np.abs(sample_f[None, :] - np.arange(in_size, dtype=np.float64)[:, None])
    w = np.maximum(0.0, 1.0 - x / kscale)
    w = w / w.sum(axis=0, keepdims=True)
    return w.T.astype(np.float32)  # [out, in]


def build_weights(inputs):
    """All packed device arrays (shared across cores)."""
    khat = _ridgelet_kernel(inputs["r_dirs"], inputs["r_scales"], inputs["r_pos"])
    # padded to 16x16 so v=15 / u out-of-range index to a zero slot
    khat_p = np.zeros((OUT_CH, 16, 16), np.float32)
    khat_p[:, :KS, :KS] = khat

    # conv lhsT: wc[p=(dv,i'), kc*512 + ch*128 + wi]
    dvip = np.arange(128)
    dv = dvip // 32
    ip = dvip % 32
    m = np.arange(512)
    ch = m // 128
    wi = m % 128
    par = ch // 2          # i parity (0=even rows, 1=odd)
    oh = ch % 2            # o half
    o = oh * 8 + wi // 16
    i2 = wi % 16
    i = 2 * i2 + par
    wc = np.zeros((128, 2048), np.float32)
    u = ip[:, None] - i[None, :] + 7          # [128, 512]
    umask = (u >= 0) & (u < KS)
    uc = np.clip(u, 0, 15)
    for kc in range(4):
        v = 4 * kc + dv                        # [128]
        vals = khat_p[o[None, :], uc, np.clip(v, 0, 15)[:, None]]
        vals = np.where(umask, vals, 0.0)
        wc[:, kc * 512:(kc + 1) * 512] = vals

    # low branch: fold resize+conv into [3072, 256], packed feature-major
    # into 24 chunks of 128 so every matmul has a full contraction dim
    A = _resize_mat()
    Ash = np.zeros((3, 8, 32), np.float32)
    for dh in range(3):
        for ph in range(8):
            r = ph + dh - 1
            if 0 <= r < 8:
                Ash[dh, ph] = A[r]
    wlow = np.asarray(inputs["wlow"], np.float32)
    # D[c,i,w,o,ph,pw] = sum_{dh,dw} wlow[o,c,dh,dw] Ash[dh,ph,i] Ash[dw,pw,w]
    D = np.einsum("ocuv,upi,vqw->ciwopq", wlow, Ash, Ash).astype(np.float32)
    D2 = D.reshape(3072, 4, 8, 8)    # [f=(c,i,w), o, ph, pw]
    W2 = np.zeros((3072, 256), np.float32)
    for g, (pp_, qq) in enumerate([(0, 0), (0, 1), (1, 0), (1, 1)]):
        W2[:, g * 64:(g + 1) * 64] = D2[:, :, pp_::2, qq::2].reshape(3072, 64)
    wlowp = W2.reshape(24, 128, 256).transpose(1, 0, 2).reshape(128, 24 * 256)

    # FC1 reorder: kstep = j2*2 + chunk over high feats, kstep 32 = low
    w1 = np.asarray(inputs["w1"], np.float32)          # [1024, 4160]
    w1hi = w1[:, 64:].reshape(1024, 16, 16, 16)        # [n, o, i2, j2]
    w1r = np.zeros((33, 128, 1024), np.float32)
    for ks in range(32):
        j2, c = ks // 2, ks % 2
        blk = w1hi[:, 8 * c:8 * (c + 1), :, j2]        # [n, 8, 16]
        w1r[ks] = blk.reshape(1024, 128).T
    w1r[32, :64, :] = w1[:, :64].T

    w2 = np.asarray(inputs["w2"], np.float32)          # [256, 1024]
    w2r = np.zeros((128, 2048), np.float32)
    for kc in range(8):
        w2r[:, kc * 256:(kc + 1) * 256] = w2[:, kc * 128:(kc + 1) * 128].T
    w3 = np.asarray(inputs["w3"], np.float32)          # [5, 256]
    w3r = np.zeros((128, 10), np.float32)
    for kc in range(2):
        w3r[:, kc * 5:(kc + 1) * 5] = w3[:, kc * 128:(kc + 1) * 128].T

    b1r = np.asarray(inputs["b1"], np.float32).reshape(8, 128).T.copy()
    b2r = np.asarray(inputs["b2"], np.float32).reshape(2, 128).T.copy()
    b3r = np.asarray(inputs["b3"], np.float32)[:, None].copy()
    blowr = np.tile(np.repeat(np.asarray(inputs["blow"], np.float32), 16),
                    2)[:, None].copy()

    return {
        "wc": wc.astype(BF16NP),
        "wlow": wlowp.astype(BF16NP),
        "w1r": w1r.astype(BF16NP),
        "w2r": w2r.astype(BF16NP),
        "w3r": w3r.astype(BF16NP),
        "b1r": np.ascontiguousarray(b1r),
        "b2r": np.ascontiguousarray(b2r),
        "b3r": b3r,
        "blowr": blowr,
    }


def pack_x(x_core):
    """[512,3,32,32] f32 -> xt2 [128, 12288] bf16, feature-major.

    xt2[p, k*512 + n] = x[n, f] for f = k*128 + p, f = c*1024 + i*32 + w.
    Low-branch rhs chunk k is the contiguous slice [:, k*512:(k+1)*512]
    with a full 128-partition contraction.
    """
    xc = np.asarray(x_core, np.float32).astype(BF16NP)
    arr = xc.reshape(BC, 3072).T.reshape(24, 128, BC)
    return np.ascontiguousarray(arr.transpose(1, 0, 2).reshape(128, 24 * BC))


def pack_x_high(x_core):
    """Channel-summed, 4x skew-replicated high-branch input, matmul-ready.

    xh[dv*32+i, t*768 + b*48 + v] = sum_c x[t*16+b, c, i, v+dv-11]
    (zero outside the image).  The conv rhs for v-chunk kc is the
    uniform-offset slice starting at 4*kc+4 of each 48-block: image col
    j = jj + (4*kc+dv) - 7.  Host-side channel sum (in fp32) shrinks the
    stream 3x and removes the on-chip adds entirely.
    """
    xc = np.asarray(x_core, np.float32)
    xs = xc.sum(axis=1).reshape(NTILES, TIMGS, 32, 32)
    buf = np.zeros((4, 32, NTILES, TIMGS, 48), np.float32)
    xsit = xs.transpose(2, 0, 1, 3)          # [i, t, b, j]
    for dv in range(4):
        lo = 11 - dv
        buf[dv, :, :, :, lo:lo + 32] = xsit
    return np.ascontiguousarray(
        buf.reshape(128, NTILES * 768).astype(BF16NP))


# ---------------------------------------------------------------- bass build
_WAIT_CARRIERS = ("InstEventSemaphore", "InstNoOp",
                  "InstUnconditionalBranch", "InstCompareAndBranch")


def _legalize_waits(nc):
    """Split excess semaphore waits onto same-engine NoOp carriers.

    The walrus codegen used by the bass2jax path allows at most 1 attached
    wait on compute instructions and 2 on DMA; Tile sometimes emits more.
    Engines execute instructions in order, so a preceding NoOp carrying the
    extra waits is equivalent.
    """
    uid = 0
    for blk in nc.m.functions[0].blocks:
        insts = blk.instructions
        i = 0
        while i < len(insts):
            inst = insts[i]
            ty = type(inst).__name__
            si = inst.sync_info
            if si is None or ty in _WAIT_CARRIERS:
                i += 1
                continue
            waits = list(si.on_wait or [])
            limit = 1
            if len(waits) <= limit:
                i += 1
                continue
            extra, keep = waits[:-limit], waits[-limit:]
            for w in extra:
                nop = mybir.InstNoOp(
                    name=f"waitnop-{uid}", engine=inst.engine,
                    sync_info=mybir.SyncInfo(on_wait=[w], on_update=[]))
                uid += 1
                insts.insert(i, nop)
                i += 1
            inst.sync_info = mybir.SyncInfo(
                on_wait=keep, on_update=list(si.on_update or []))
            i += 1


def build_nc(skip_conv=False, skip_low=False, skip_fc1=False, skip_fc23=False):
    nc = bass.Bass()
    xt_d = nc.declare_dram_parameter("xt", [128, 24 * BC], BF16, isOutput=False)
    xth_d = nc.declare_dram_parameter("xth", [128, NTILES * 768], BF16, isOutput=False)
    wc_d = nc.declare_dram_parameter("wc", [128, 2048], BF16, isOutput=False)
    wlow_d = nc.declare_dram_parameter("wlow", [128, 24 * 256], BF16, isOutput=False)
    w1_d = nc.declare_dram_parameter("w1r", [33, 128, 1024], BF16, isOutput=False)
    w2_d = nc.declare_dram_parameter("w2r", [128, 2048], BF16, isOutput=False)
    w3_d = nc.declare_dram_parameter("w3r", [128, 10], BF16, isOutput=False)
    b1_d = nc.declare_dram_parameter("b1r", [128, 8], FP32, isOutput=False)
    b2_d = nc.declare_dram_parameter("b2r", [128, 2], FP32, isOutput=False)
    b3_d = nc.declare_dram_parameter("b3r", [5, 1], FP32, isOutput=False)
    bl_d = nc.declare_dram_parameter("blowr", [128, 1], FP32, isOutput=False)
    y_d = nc.declare_dram_parameter("y", [5, 512], FP32, isOutput=True)

    RELU = mybir.ActivationFunctionType.Relu

    with tile.TileContext(nc) as tc:
        with (
            tc.tile_pool(name="persist", bufs=1) as pp,
            tc.tile_pool(name="work", bufs=3) as wp,
            tc.tile_pool(name="w1pool", bufs=4) as w1p,
            # single rotating PSUM pool for every phase: pool-close
            # barriers between phases cost 2-3us each; one pool turns
            # them into per-slot WAR deps (all tiles are [128,512] f32)
            tc.tile_pool(name="allpsum", bufs=8, space="PSUM") as cps,
        ):
            # wc first: the first conv matmul only needs wc + xh[0].  The
            # 3MB tball / wlow / w2 are not needed until ~120us in, and
            # their transfers would steal HBM bandwidth from the xh
            # stream, so they are issued from the (busy) scalar engine
            # inside the conv loop to physically delay them.
            # one tile per kc chunk: tile-granular deps let the kc=0
            # matmuls start as soon as their own chunk lands
            wc_sb = [pp.tile([128, 512], BF16, tag=f"wc{q}", name=f"wc{q}")
                     for q in range(4)]
            for q, eng in enumerate((nc.gpsimd, nc.sync, nc.scalar, nc.scalar)):
                eng.dma_start(out=wc_sb[q][:],
                              in_=wc_d[:, q * 512:(q + 1) * 512])
            b1_sb = pp.tile([128, 8], FP32, tag="b1")
            nc.sync.dma_start(out=b1_sb[:], in_=b1_d[:])
            b2_sb = pp.tile([128, 2], FP32, tag="b2")
            nc.sync.dma_start(out=b2_sb[:], in_=b2_d[:])
            b3_sb = pp.tile([5, 1], FP32, tag="b3")
            nc.sync.dma_start(out=b3_sb[:], in_=b3_d[:])
            bl_sb = pp.tile([128, 1], FP32, tag="bl")
            nc.sync.dma_start(out=bl_sb[:], in_=bl_d[:])
            w3_sb = pp.tile([128, 10], BF16, tag="w3")
            nc.sync.dma_start(out=w3_sb[:], in_=w3_d[:])
            w2_sb = pp.tile([128, 2048], BF16, tag="w2")
            wlow_sb = pp.tile([128, 24 * 256], BF16, tag="wlow")
            tball = pp.tile([128, 24 * BC], BF16, tag="tball")

            ph0 = pp.tile([128, 8192], BF16, tag="ph0")
            ph1 = pp.tile([128, 8192], BF16, tag="ph1")
            xlow_sb = pp.tile([128, 512], BF16, tag="xlow")
            nc.vector.memset(xlow_sb[:], 0.0)
            # per-chunk tiles: tile-granular deps let FC2/FC3 start on
            # chunk 0 without waiting for every activate
            h1_sb = [pp.tile([128, 512], BF16, tag=f"h1_{i}", name=f"h1_{i}")
                     for i in range(8)]
            h2_sb = [pp.tile([128, 512], BF16, tag=f"h2_{i}", name=f"h2_{i}")
                     for i in range(2)]
            y_sb = pp.tile([5, 512], FP32, tag="ysb")

            # ---------------- conv + pool over 32 tiles
            if True:
                for t in range(0 if skip_conv else NTILES):
                    xh = wp.tile([128, 768], BF16, tag="xh", bufs=10)
                    nc.gpsimd.dma_start(
                        out=xh[:], in_=xth_d[:, t * 768:(t + 1) * 768])
                    # late-issued persistent loads (see note above)
                    if t in (8, 11, 14, 17):
                        q = (t - 8) // 3
                        sl = slice(q * 3072, (q + 1) * 3072)
                        nc.scalar.dma_start(out=tball[:, sl], in_=xt_d[:, sl])
                    elif t == 20:
                        nc.scalar.dma_start(out=wlow_sb[:], in_=wlow_d[:])
                    elif t == 23:
                        nc.scalar.dma_start(out=w2_sb[:], in_=w2_d[:])

                    xhv = xh[:].rearrange("p (b v) -> p b v", v=48)
                    cp = [cps.tile([128, 512], FP32, tag="ps", name=f"cp{t}_{i}") for i in range(4)]
                    for kc in range(4):
                        off = 4 * kc + 4
                        rhs = xhv[:, :, off:off + 32]
                        for mc in range(4):
                            nc.tensor.matmul(
                                cp[mc][:],
                                wc_sb[kc][:, mc * 128:(mc + 1) * 128],
                                rhs,
                                start=(kc == 0),
                                stop=(kc == 3),
                            )
                    # relu first (1-input PSUM read), then pool in SBUF bf16
                    s = [wp.tile([128, 512], BF16, tag=f"s{i}", name=f"s{t}_{i}")
                         for i in range(4)]
                    for i in range(4):
                        nc.scalar.activation(out=s[i][:], in_=cp[i][:], func=RELU)
                    m0 = wp.tile([128, 512], BF16, tag="m0")
                    m1 = wp.tile([128, 512], BF16, tag="m1")
                    nc.vector.tensor_max(out=m0[:], in0=s[0][:], in1=s[2][:])
                    nc.vector.tensor_max(out=m1[:], in0=s[1][:], in1=s[3][:])
                    # pooled feats land in (j2, t, b) free order so FC1 rhs
                    # slices are contiguous; iterate (j, b) with contiguous
                    # 16-elem b-runs on the write side
                    m0v = m0[:].rearrange("p (b j t) -> p j b t", j=16, t=2)
                    m1v = m1[:].rearrange("p (b j t) -> p j b t", j=16, t=2)
                    ph0v_o = ph0[:].rearrange(
                        "p (j t b) -> p t j b", t=NTILES, b=TIMGS)[:, t]
                    ph1v_o = ph1[:].rearrange(
                        "p (j t b) -> p t j b", t=NTILES, b=TIMGS)[:, t]
                    nc.vector.tensor_max(
                        out=ph0v_o, in0=m0v[:, :, :, 0], in1=m0v[:, :, :, 1])
                    nc.vector.tensor_max(
                        out=ph1v_o, in0=m1v[:, :, :, 0], in1=m1v[:, :, :, 1])

            # ---------------- low branch (contiguous rhs slices, m=128)
            if True:
                lp = [cps.tile([128, 512], FP32, tag="ps", name=f"lp{g}")
                      for g in range(2)]
                for wg in range(0 if skip_low else 24):
                    rhs = tball[:, wg * 512:(wg + 1) * 512]
                    for g in range(2):
                        nc.tensor.matmul(
                            lp[g][:],
                            wlow_sb[:, wg * 256 + g * 128: wg * 256 + (g + 1) * 128],
                            rhs, start=(wg == 0), stop=(wg == 23))
                # relu(x+bias) per group first, then pool in SBUF; the
                # final max crosses partition halves, so DMA-realign the
                # top half (TensorTensor needs equal base partitions)
                r = [wp.tile([128, 512], BF16, tag=f"r{g}", name=f"r{g}")
                     for g in range(2)]
                for g in range(2):
                    nc.scalar.activation(
                        out=r[g][:], in_=lp[g][:], func=RELU, bias=bl_sb[:, 0:1])
                q0 = wp.tile([128, 512], BF16, tag="q0")
                nc.vector.tensor_max(out=q0[:], in0=r[0][:], in1=r[1][:])
                qb = wp.tile([64, 512], BF16, tag="qb")
                nc.sync.dma_start(out=qb[:], in_=q0[64:128, :])
                nc.vector.tensor_max(
                    out=xlow_sb[0:64, :], in0=q0[0:64, :], in1=qb[:])

            # ---------------- FC1 (weights streamed)
            if True:
                fp = [cps.tile([128, 512], FP32, tag="ps", name=f"fp{i}") for i in range(8)]
                for ks in range(0 if skip_fc1 else 33):
                    w1t = w1p.tile([128, 1024], BF16, tag="w1t", bufs=12)
                    # alternate rings: one ring can't sustain the 154 GB/s
                    # the FC1 weight stream needs.  scalar (not sync) for
                    # the odd tiles: its engine only reaches these issue
                    # points at FC1 time, so the stream can't collide with
                    # the startup-critical wc transfers on sync.
                    w1_eng = nc.gpsimd if ks % 2 == 0 else nc.scalar
                    w1_eng.dma_start(out=w1t[:], in_=w1_d[ks])
                    if ks < 32:
                        j2, c = ks // 2, ks % 2
                        src = ph0 if c == 0 else ph1
                        rhs = src[:, j2 * 512:(j2 + 1) * 512]
                    else:
                        rhs = xlow_sb[:]
                    for mc in range(8):
                        nc.tensor.matmul(
                            fp[mc][:], w1t[:, mc * 128:(mc + 1) * 128], rhs,
                            start=(ks == 0), stop=(ks == 32))
                for mc in range(8):
                    nc.scalar.activation(
                        out=h1_sb[mc][:], in_=fp[mc][:],
                        func=RELU, bias=b1_sb[:, mc:mc + 1])

            # ---------------- FC2 + FC3
            if True:
                gp = [cps.tile([128, 512], FP32, tag="ps", name=f"gp{i}") for i in range(2)]
                for kc in range(0 if skip_fc23 else 8):
                    rhs = h1_sb[kc][:]
                    for mc in range(2):
                        nc.tensor.matmul(
                            gp[mc][:],
                            w2_sb[:, kc * 256 + mc * 128: kc * 256 + (mc + 1) * 128],
                            rhs, start=(kc == 0), stop=(kc == 7))
                for mc in range(2):
                    nc.scalar.activation(
                        out=h2_sb[mc][:], in_=gp[mc][:],
                        func=RELU, bias=b2_sb[:, mc:mc + 1])

                yp = cps.tile([128, 512], FP32, tag="ps", name="yp")
                for kc in range(2):
                    nc.tensor.matmul(
                        yp[0:5, :], w3_sb[:, kc * 5:(kc + 1) * 5],
                        h2_sb[kc][:],
                        start=(kc == 0), stop=(kc == 1))
                nc.vector.tensor_scalar_add(
                    out=y_sb[:], in0=yp[0:5, :], scalar1=b3_sb[:, 0:1])
                nc.sync.dma_start(out=y_d[:], in_=y_sb[:])

    _legalize_waits(nc)
    return nc


_NC = None
TRACE = False
LAST_RESULTS = None


def kernel(**inputs):
    global _NC, LAST_RESULTS
    w = build_weights(inputs)
    if _NC is None:
        _NC = build_nc()
    x = np.asarray(inputs["x"], np.float32)
    in_maps = []
    for c in range(NCORES):
        xs_c = x[c * BC:(c + 1) * BC]
        m = {"xt": pack_x(xs_c), "xth": pack_x_high(xs_c)}
        m.update(w)
        in_maps.append(m)
    res = run_bass_kernel_spmd(_NC, in_maps, list(range(NCORES)), trace=TRACE)
    LAST_RESULTS = res
    y = np.concatenate(
        [np.asarray(res.results[i]["y"], np.float32).T for i in range(NCORES)], axis=0)
    return y



# revision 42
# speedup vs baseline: 1.0172x; 1.0042x over previous
"""Trainium2 Bass kernel for nn_MixedResolutionCNN.

Network (per sample, eval mode):
  high branch: ridgelet conv 3->16 k=15 same-pad (kernel broadcast over in-ch)
               -> relu -> maxpool2 -> 4096 feats
  low branch:  bilinear resize 32->8 -> conv 3->4 k=3 pad1 + bias -> relu
               -> maxpool2 -> 64 feats
  head:        concat -> fc 4160->1024 relu -> 1024->256 relu -> 256->5

Device strategy (pure data parallel over 8 cores, 512 images/core):

* The ridgelet kernel is identical across the 3 input channels, so the high
  conv contracts the channel-summed image xs = sum_c x[:,c] with a 16x15x15
  kernel. Expressed as matmuls with contraction over (v, i') = (kernel col,
  image row): out[(o,i),(b,j)] = sum Khat[o, i'-i+7, v] * xs[b, i', j+v-7].
  The host pre-sums channels (fp32) and ships a skewed 4x replication:
  partition block dv holds xs shifted by dv columns, so the moving operand
  for v-chunk kc is the uniform-offset slice [4kc+4, +32) of each 48-col
  image block - matmul-ready straight off the DMA, no on-chip prep.
  4 K-chunks x 4 M-chunks of [128,128,512] matmuls per 16-image tile.
* PE moving-operand APs must be innermost-contiguous: strided rhs runs the
  PE at 1/4 rate (measured).  All layouts here are chosen so every rhs is
  either contiguous 512 or contiguous-32-in-48 blocks.
* relu/maxpool: relu on scalar from PSUM, 2x2 pool as elementwise maxes on
  vector (relu commutes with max). Pooled features land in (j2, t, b) free
  order so every FC1 rhs slice is contiguous; no transpose anywhere.
* low branch: resize+conv fold into one linear map [3072, 256], packed
  feature-major as 24 full-128 contraction chunks whose rhs are contiguous
  slices of the f-major image tile; pool crosses partition halves via one
  64KB SBUF-SBUF DMA realign.
* DMA ring scheduling: the first conv matmuls only need their own wc chunk
  (4 tiles, one per kc, spread across rings); tball/wlow/w2 are issued from
  the busy scalar engine inside the conv loop to delay them off the
  startup-critical HBM window; the FC1 weight stream (8.5MB bf16) runs
  double-buffered alternating gpsimd/scalar rings (one ring tops out
  ~126GB/s, the stream needs 154GB/s).
"""

import numpy as np
import ml_dtypes

import concourse.bass as bass
import concourse.tile as tile
from concourse import mybir
from concourse.bass_utils import run_bass_kernel_spmd

BF16NP = ml_dtypes.bfloat16
FP32 = mybir.dt.float32
BF16 = mybir.dt.bfloat16

B = 4096
NCORES = 8
BC = B // NCORES           # 512 images per core
TIMGS = 16                 # images per tile
NTILES = BC // TIMGS       # 32
KS = 15
OUT_CH = 16


# ---------------------------------------------------------------- host math
def _ridgelet_kernel(r_dirs, r_scales, r_pos):
    """[16,15,15] channel-shared ridgelet kernel, mirrors reference."""
    c = np.arange(KS, dtype=np.float32) - KS // 2
    x1 = c[:, None]
    x2 = c[None, :]
    d = np.asarray(r_dirs, np.float32)[:, None, None]
    s = np.asarray(r_scales, np.float32)[:, None, None]
    p = np.asarray(r_pos, np.float32)[:, None, None]
    t = (x1 * np.cos(d) + x2 * np.sin(d) - p) / s
    vals = np.exp(-t * t / 2.0) - 0.5 * np.exp(-t * t / 8.0)
    return vals.reshape(OUT_CH, 10, KS, KS).sum(axis=1)


def _resize_mat(in_size=32, out_size=8):
    """Row matrix of jax.image.resize(..., 'bilinear', antialias=True)."""
    scale = out_size / in_size
    inv = 1.0 / scale
    kscale = max(inv, 1.0)
    sample_f = (np.arange(out_size, dtype=np.float64) + 0.5) * inv - 0.5
    x = np.abs(sample_f[None, :] - np.arange(in_size, dtype=np.float64)[:, None])
    w = np.maximum(0.0, 1.0 - x / kscale)
    w = w / w.sum(axis=0, keepdims=True)
    return w.T.astype(np.float32)  # [out, in]


def build_weights(inputs):
    """All packed device arrays (shared across cores)."""
    khat = _ridgelet_kernel(inputs["r_dirs"], inputs["r_scales"], inputs["r_pos"])
    # padded to 16x16 so v=15 / u out-of-range index to a zero slot
    khat_p = np.zeros((OUT_CH, 16, 16), np.float32)
    khat_p[:, :KS, :KS] = khat

    # conv lhsT: wc[p=(dv,i'), kc*512 + ch*128 + wi]
    dvip = np.arange(128)
    dv = dvip // 32
    ip = dvip % 32
    m = np.arange(512)
    ch = m // 128
    wi = m % 128
    par = ch // 2          # i parity (0=even rows, 1=odd)
    oh = ch % 2            # o half
    o = oh * 8 + wi // 16
    i2 = wi % 16
    i = 2 * i2 + par
    wc = np.zeros((128, 2048), np.float32)
    u = ip[:, None] - i[None, :] + 7          # [128, 512]
    umask = (u >= 0) & (u < KS)
    uc = np.clip(u, 0, 15)
    for kc in range(4):
        v = 4 * kc + dv                        # [128]
        vals = khat_p[o[None, :], uc, np.clip(v, 0, 15)[:, None]]
        vals = np.where(umask, vals, 0.0)
        wc[:, kc * 512:(kc + 1) * 512] = vals

    # low branch: fold resize+conv into [3072, 256], packed feature-major
    # into 24 chunks of 128 so every matmul has a full contraction dim
    A = _resize_mat()
    Ash = np.zeros((3, 8, 32), np.float32)
    for dh in range(3):
        for ph in range(8):
            r = ph + dh - 1
            if 0 <= r < 8:
                Ash[dh, ph] = A[r]
    wlow = np.asarray(inputs["wlow"], np.float32)
    # D[c,i,w,o,ph,pw] = sum_{dh,dw} wlow[o,c,dh,dw] Ash[dh,ph,i] Ash[dw,pw,w]
    D = np.einsum("ocuv,upi,vqw->ciwopq", wlow, Ash, Ash).astype(np.float32)
    D2 = D.reshape(3072, 4, 8, 8)    # [f=(c,i,w), o, ph, pw]
    W2 = np.zeros((3072, 256), np.float32)
    for g, (pp_, qq) in enumerate([(0, 0), (0, 1), (1, 0), (1, 1)]):
        W2[:, g * 64:(g + 1) * 64] = D2[:, :, pp_::2, qq::2].reshape(3072, 64)
    wlowp = W2.reshape(24, 128, 256).transpose(1, 0, 2).reshape(128, 24 * 256)

    # FC1 reorder: kstep = j2*2 + chunk over high feats, kstep 32 = low
    w1 = np.asarray(inputs["w1"], np.float32)          # [1024, 4160]
    w1hi = w1[:, 64:].reshape(1024, 16, 16, 16)        # [n, o, i2, j2]
    w1r = np.zeros((33, 128, 1024), np.float32)
    for ks in range(32):
        j2, c = ks // 2, ks % 2
        blk = w1hi[:, 8 * c:8 * (c + 1), :, j2]        # [n, 8, 16]
        w1r[ks] = blk.reshape(1024, 128).T
    w1r[32, :64, :] = w1[:, :64].T

    w2 = np.asarray(inputs["w2"], np.float32)          # [256, 1024]
    w2r = np.zeros((128, 2048), np.float32)
    for kc in range(8):
        w2r[:, kc * 256:(kc + 1) * 256] = w2[:, kc * 128:(kc + 1) * 128].T
    w3 = np.asarray(inputs["w3"], np.float32)          # [5, 256]
    w3r = np.zeros((128, 10), np.float32)
    for kc in range(2):
        w3r[:, kc * 5:(kc + 1) * 5] = w3[:, kc * 128:(kc + 1) * 128].T

    b1r = np.asarray(inputs["b1"], np.float32).reshape(8, 128).T.copy()
    b2r = np.asarray(inputs["b2"], np.float32).reshape(2, 128).T.copy()
    b3r = np.asarray(inputs["b3"], np.float32)[:, None].copy()
    blowr = np.tile(np.repeat(np.asarray(inputs["blow"], np.float32), 16),
                    2)[:, None].copy()

    return {
        "wc": wc.astype(BF16NP),
        "wlow": wlowp.astype(BF16NP),
        "w1r": w1r.astype(BF16NP),
        "w2r": w2r.astype(BF16NP),
        "w3r": w3r.astype(BF16NP),
        "b1r": np.ascontiguousarray(b1r),
        "b2r": np.ascontiguousarray(b2r),
        "b3r": b3r,
        "blowr": blowr,
    }


def pack_x(x_core):
    """[512,3,32,32] f32 -> xt2 [128, 12288] bf16, feature-major.

    xt2[p, k*512 + n] = x[n, f] for f = k*128 + p, f = c*1024 + i*32 + w.
    Low-branch rhs chunk k is the contiguous slice [:, k*512:(k+1)*512]
    with a full 128-partition contraction.
    """
    xc = np.asarray(x_core, np.float32).astype(BF16NP)
    arr = xc.reshape(BC, 3072).T.reshape(24, 128, BC)
    return np.ascontiguousarray(arr.transpose(1, 0, 2).reshape(128, 24 * BC))


def pack_x_high(x_core):
    """Channel-summed, 4x skew-replicated high-branch input, matmul-ready.

    xh[dv*32+i, t*768 + b*48 + v] = sum_c x[t*16+b, c, i, v+dv-11]
    (zero outside the image).  The conv rhs for v-chunk kc is the
    uniform-offset slice starting at 4*kc+4 of each 48-block: image col
    j = jj + (4*kc+dv) - 7.  Host-side channel sum (in fp32) shrinks the
    stream 3x and removes the on-chip adds entirely.
    """
    xc = np.asarray(x_core, np.float32)
    xs = xc.sum(axis=1).reshape(NTILES, TIMGS, 32, 32)
    buf = np.zeros((4, 32, NTILES, TIMGS, 48), np.float32)
    xsit = xs.transpose(2, 0, 1, 3)          # [i, t, b, j]
    for dv in range(4):
        lo = 11 - dv
        buf[dv, :, :, :, lo:lo + 32] = xsit
    return np.ascontiguousarray(
        buf.reshape(128, NTILES * 768).astype(BF16NP))


# ---------------------------------------------------------------- bass build
_WAIT_CARRIERS = ("InstEventSemaphore", "InstNoOp",
                  "InstUnconditionalBranch", "InstCompareAndBranch")


def _legalize_waits(nc):
    """Split excess semaphore waits onto same-engine NoOp carriers.

    The walrus codegen used by the bass2jax path allows at most 1 attached
    wait on compute instructions and 2 on DMA; Tile sometimes emits more.
    Engines execute instructions in order, so a preceding NoOp carrying the
    extra waits is equivalent.
    """
    uid = 0
    for blk in nc.m.functions[0].blocks:
        insts = blk.instructions
        i = 0
        while i < len(insts):
            inst = insts[i]
            ty = type(inst).__name__
            si = inst.sync_info
            if si is None or ty in _WAIT_CARRIERS:
                i += 1
                continue
            waits = list(si.on_wait or [])
            limit = 1
            if len(waits) <= limit:
                i += 1
                continue
            extra, keep = waits[:-limit], waits[-limit:]
            for w in extra:
                nop = mybir.InstNoOp(
                    name=f"waitnop-{uid}", engine=inst.engine,
                    sync_info=mybir.SyncInfo(on_wait=[w], on_update=[]))
                uid += 1
                insts.insert(i, nop)
                i += 1
            inst.sync_info = mybir.SyncInfo(
                on_wait=keep, on_update=list(si.on_update or []))
            i += 1


def build_nc(skip_conv=False, skip_low=False, skip_fc1=False, skip_fc23=False):
    nc = bass.Bass()
    xt_d = nc.declare_dram_parameter("xt", [128, 24 * BC], BF16, isOutput=False)
    xth_d = nc.declare_dram_parameter("xth", [128, NTILES * 768], BF16, isOutput=False)
    wc_d = nc.declare_dram_parameter("wc", [128, 2048], BF16, isOutput=False)
    wlow_d = nc.declare_dram_parameter("wlow", [128, 24 * 256], BF16, isOutput=False)
    w1_d = nc.declare_dram_parameter("w1r", [33, 128, 1024], BF16, isOutput=False)
    w2_d = nc.declare_dram_parameter("w2r", [128, 2048], BF16, isOutput=False)
    w3_d = nc.declare_dram_parameter("w3r", [128, 10], BF16, isOutput=False)
    b1_d = nc.declare_dram_parameter("b1r", [128, 8], FP32, isOutput=False)
    b2_d = nc.declare_dram_parameter("b2r", [128, 2], FP32, isOutput=False)
    b3_d = nc.declare_dram_parameter("b3r", [5, 1], FP32, isOutput=False)
    bl_d = nc.declare_dram_parameter("blowr", [128, 1], FP32, isOutput=False)
    y_d = nc.declare_dram_parameter("y", [5, 512], FP32, isOutput=True)

    RELU = mybir.ActivationFunctionType.Relu

    with tile.TileContext(nc) as tc:
        with (
            tc.tile_pool(name="persist", bufs=1) as pp,
            tc.tile_pool(name="work", bufs=3) as wp,
            tc.tile_pool(name="w1pool", bufs=4) as w1p,
            # single rotating PSUM pool for every phase: pool-close
            # barriers between phases cost 2-3us each; one pool turns
            # them into per-slot WAR deps (all tiles are [128,512] f32)
            tc.tile_pool(name="allpsum", bufs=8, space="PSUM") as cps,
        ):
            # wc first: the first conv matmul only needs wc + xh[0].  The
            # 3MB tball / wlow / w2 are not needed until ~120us in, and
            # their transfers would steal HBM bandwidth from the xh
            # stream, so they are issued from the (busy) scalar engine
            # inside the conv loop to physically delay them.
            # one tile per kc chunk: tile-granular deps let the kc=0
            # matmuls start as soon as their own chunk lands
            wc_sb = [pp.tile([128, 512], BF16, tag=f"wc{q}", name=f"wc{q}")
                     for q in range(4)]
            for q, eng in enumerate((nc.sync, nc.sync, nc.scalar, nc.scalar)):
                eng.dma_start(out=wc_sb[q][:],
                              in_=wc_d[:, q * 512:(q + 1) * 512])
            b1_sb = pp.tile([128, 8], FP32, tag="b1")
            nc.sync.dma_start(out=b1_sb[:], in_=b1_d[:])
            b2_sb = pp.tile([128, 2], FP32, tag="b2")
            nc.sync.dma_start(out=b2_sb[:], in_=b2_d[:])
            b3_sb = pp.tile([5, 1], FP32, tag="b3")
            nc.sync.dma_start(out=b3_sb[:], in_=b3_d[:])
            bl_sb = pp.tile([128, 1], FP32, tag="bl")
            nc.sync.dma_start(out=bl_sb[:], in_=bl_d[:])
            w3_sb = pp.tile([128, 10], BF16, tag="w3")
            nc.sync.dma_start(out=w3_sb[:], in_=w3_d[:])
            w2_sb = pp.tile([128, 2048], BF16, tag="w2")
            wlow_sb = pp.tile([128, 24 * 256], BF16, tag="wlow")
            tball = pp.tile([128, 24 * BC], BF16, tag="tball")

            ph0 = pp.tile([128, 8192], BF16, tag="ph0")
            ph1 = pp.tile([128, 8192], BF16, tag="ph1")
            xlow_sb = pp.tile([128, 512], BF16, tag="xlow")
            nc.vector.memset(xlow_sb[:], 0.0)
            # per-chunk tiles: tile-granular deps let FC2/FC3 start on
            # chunk 0 without waiting for every activate
            h1_sb = [pp.tile([128, 512], BF16, tag=f"h1_{i}", name=f"h1_{i}")
                     for i in range(8)]
            h2_sb = [pp.tile([128, 512], BF16, tag=f"h2_{i}", name=f"h2_{i}")
                     for i in range(2)]
            y_sb = pp.tile([5, 512], FP32, tag="ysb")

            # ---------------- conv + pool over 32 tiles
            if True:
                for t in range(0 if skip_conv else NTILES):
                    xh = wp.tile([128, 768], BF16, tag="xh", bufs=10)
                    nc.gpsimd.dma_start(
                        out=xh[:], in_=xth_d[:, t * 768:(t + 1) * 768])
                    # late-issued persistent loads (see note above)
                    if t in (8, 11, 14, 17):
                        q = (t - 8) // 3
                        sl = slice(q * 3072, (q + 1) * 3072)
                        nc.scalar.dma_start(out=tball[:, sl], in_=xt_d[:, sl])
                    elif t == 20:
                        nc.scalar.dma_start(out=wlow_sb[:], in_=wlow_d[:])
                    elif t == 23:
                        nc.scalar.dma_start(out=w2_sb[:], in_=w2_d[:])

                    xhv = xh[:].rearrange("p (b v) -> p b v", v=48)
                    cp = [cps.tile([128, 512], FP32, tag="ps", name=f"cp{t}_{i}") for i in range(4)]
                    for kc in range(4):
                        off = 4 * kc + 4
                        rhs = xhv[:, :, off:off + 32]
                        for mc in range(4):
                            nc.tensor.matmul(
                                cp[mc][:],
                                wc_sb[kc][:, mc * 128:(mc + 1) * 128],
                                rhs,
                                start=(kc == 0),
                                stop=(kc == 3),
                            )
                    # relu first (1-input PSUM read), then pool in SBUF bf16
                    s = [wp.tile([128, 512], BF16, tag=f"s{i}", name=f"s{t}_{i}")
                         for i in range(4)]
                    for i in range(4):
                        nc.scalar.activation(out=s[i][:], in_=cp[i][:], func=RELU)
                    m0 = wp.tile([128, 512], BF16, tag="m0")
                    m1 = wp.tile([128, 512], BF16, tag="m1")
                    nc.vector.tensor_max(out=m0[:], in0=s[0][:], in1=s[2][:])
                    nc.vector.tensor_max(out=m1[:], in0=s[1][:], in1=s[3][:])
                    # pooled feats land in (j2, t, b) free order so FC1 rhs
                    # slices are contiguous; iterate (j, b) with contiguous
                    # 16-elem b-runs on the write side
                    m0v = m0[:].rearrange("p (b j t) -> p j b t", j=16, t=2)
                    m1v = m1[:].rearrange("p (b j t) -> p j b t", j=16, t=2)
                    ph0v_o = ph0[:].rearrange(
                        "p (j t b) -> p t j b", t=NTILES, b=TIMGS)[:, t]
                    ph1v_o = ph1[:].rearrange(
                        "p (j t b) -> p t j b", t=NTILES, b=TIMGS)[:, t]
                    nc.vector.tensor_max(
                        out=ph0v_o, in0=m0v[:, :, :, 0], in1=m0v[:, :, :, 1])
                    nc.vector.tensor_max(
                        out=ph1v_o, in0=m1v[:, :, :, 0], in1=m1v[:, :, :, 1])

            # ---------------- low branch (contiguous rhs slices, m=128)
            if True:
                lp = [cps.tile([128, 512], FP32, tag="ps", name=f"lp{g}")
                      for g in range(2)]
                for wg in range(0 if skip_low else 24):
                    rhs = tball[:, wg * 512:(wg + 1) * 512]
                    for g in range(2):
                        nc.tensor.matmul(
                            lp[g][:],
                            wlow_sb[:, wg * 256 + g * 128: wg * 256 + (g + 1) * 128],
                            rhs, start=(wg == 0), stop=(wg == 23))
                # relu(x+bias) per group first, then pool in SBUF; the
                # final max crosses partition halves, so DMA-realign the
                # top half (TensorTensor needs equal base partitions)
                r = [wp.tile([128, 512], BF16, tag=f"r{g}", name=f"r{g}")
                     for g in range(2)]
                for g in range(2):
                    nc.scalar.activation(
                        out=r[g][:], in_=lp[g][:], func=RELU, bias=bl_sb[:, 0:1])
                q0 = wp.tile([128, 512], BF16, tag="q0")
                nc.vector.tensor_max(out=q0[:], in0=r[0][:], in1=r[1][:])
                qb = wp.tile([64, 512], BF16, tag="qb")
                nc.sync.dma_start(out=qb[:], in_=q0[64:128, :])
                nc.vector.tensor_max(
                    out=xlow_sb[0:64, :], in0=q0[0:64, :], in1=qb[:])

            # ---------------- FC1 (weights streamed)
            if True:
                fp = [cps.tile([128, 512], FP32, tag="ps", name=f"fp{i}") for i in range(8)]
                for ks in range(0 if skip_fc1 else 33):
                    w1t = w1p.tile([128, 1024], BF16, tag="w1t", bufs=12)
                    # alternate rings: one ring can't sustain the 154 GB/s
                    # the FC1 weight stream needs.  scalar (not sync) for
                    # the odd tiles: its engine only reaches these issue
                    # points at FC1 time, so the stream can't collide with
                    # the startup-critical wc transfers on sync.
                    w1_eng = nc.gpsimd if ks % 2 == 0 else nc.scalar
                    w1_eng.dma_start(out=w1t[:], in_=w1_d[ks])
                    if ks < 32:
                        j2, c = ks // 2, ks % 2
                        src = ph0 if c == 0 else ph1
                        rhs = src[:, j2 * 512:(j2 + 1) * 512]
                    else:
                        rhs = xlow_sb[:]
                    for mc in range(8):
                        nc.tensor.matmul(
                            fp[mc][:], w1t[:, mc * 128:(mc + 1) * 128], rhs,
                            start=(ks == 0), stop=(ks == 32))
                for mc in range(8):
                    nc.scalar.activation(
                        out=h1_sb[mc][:], in_=fp[mc][:],
                        func=RELU, bias=b1_sb[:, mc:mc + 1])

            # ---------------- FC2 + FC3
            if True:
                gp = [cps.tile([128, 512], FP32, tag="ps", name=f"gp{i}") for i in range(2)]
                for kc in range(0 if skip_fc23 else 8):
                    rhs = h1_sb[kc][:]
                    for mc in range(2):
                        nc.tensor.matmul(
                            gp[mc][:],
                            w2_sb[:, kc * 256 + mc * 128: kc * 256 + (mc + 1) * 128],
                            rhs, start=(kc == 0), stop=(kc == 7))
                for mc in range(2):
                    nc.scalar.activation(
                        out=h2_sb[mc][:], in_=gp[mc][:],
                        func=RELU, bias=b2_sb[:, mc:mc + 1])

                yp = cps.tile([128, 512], FP32, tag="ps", name="yp")
                for kc in range(2):
                    nc.tensor.matmul(
                        yp[0:5, :], w3_sb[:, kc * 5:(kc + 1) * 5],
                        h2_sb[kc][:],
                        start=(kc == 0), stop=(kc == 1))
                nc.vector.tensor_scalar_add(
                    out=y_sb[:], in0=yp[0:5, :], scalar1=b3_sb[:, 0:1])
                nc.sync.dma_start(out=y_d[:], in_=y_sb[:])

    _legalize_waits(nc)
    return nc


_NC = None
TRACE = False
LAST_RESULTS = None


def kernel(**inputs):
    global _NC, LAST_RESULTS
    w = build_weights(inputs)
    if _NC is None:
        _NC = build_nc()
    x = np.asarray(inputs["x"], np.float32)
    in_maps = []
    for c in range(NCORES):
        xs_c = x[c * BC:(c + 1) * BC]
        m = {"xt": pack_x(xs_c), "xth": pack_x_high(xs_c)}
        m.update(w)
        in_maps.append(m)
    res = run_bass_kernel_spmd(_NC, in_maps, list(range(NCORES)), trace=TRACE)
    LAST_RESULTS = res
    y = np.concatenate(
        [np.asarray(res.results[i]["y"], np.float32).T for i in range(NCORES)], axis=0)
    return y

